# revision 3
# baseline (speedup 1.0000x reference)
"""Multi-head attention (B=2, S=4096, D=512, H=8) on 8 Trainium2 NeuronCores.

Sharding: batch x head-pair.  Core c handles batch b = c//4 and heads
(2*(c%4), 2*(c%4)+1).  Each core computes its heads' Q/K/V projections,
flash-style attention (scores kept transposed [kv, q] so the attn@V matmul
consumes the exp() output directly, with softmax denominators accumulated via
an extra ones-column on V), and its heads' slice of the output projection.
The 4 per-batch partial outputs are summed on the host (row-parallel linear)
and the output bias is added there.

exp() is split across engines: the Activation engine computes exact exp for
2/3 of the score slices (two instructions per event, so PSUM banks free up at
slice granularity), and the Vector engine computes the remaining 1/3 with a
one-instruction Schraudolph bit-trick exp (scale+bias into int16, bitcast to
bf16).  Softmax's normalization cancels the correlated part of the
approximation error; measured end-to-end rel-err ~8.6e-3 vs 4.6e-3 all-exact.

All matmul operands are bf16 (f32 PSUM accumulation); inputs are transposed
and cast on the host so the device consumes [d, s]-layout activations
directly (no on-device transposes).
"""

import sys

sys.path.insert(0, "/opt/trn_rl_repo")

import numpy as np
import ml_dtypes

import concourse.bacc as bacc
import concourse.bass as bass
import concourse.tile as tile
from concourse import mybir
from concourse.bass_utils import run_bass_kernel_spmd

BF16 = ml_dtypes.bfloat16

B = 2
S = 4096
D = 512
H = 8
DH = 64           # head dim
HPC = 2           # heads per core
D2 = HPC * DH     # 128, the two heads' feature slice
N_CORES = 8
QC = 512          # query chunk (free dim of scores/attnV matmuls)
KVC = 128         # kv chunk (partition dim of transposed scores)
N_QC = S // QC    # 8
N_KVC = S // KVC  # 32
GROUP = 3         # kv chunks per event (PSUM banks per S tile)

FP32 = mybir.dt.float32
BF16_T = mybir.dt.bfloat16
I16 = mybir.dt.int16
AF = mybir.ActivationFunctionType
ALU = mybir.AluOpType

# Schraudolph exp constants for the DVE path: bitcast_bf16(int16(
#   score * (0.125*log2(e)*128) + (127*128 - C))) ~= exp(score * 0.125).
# C=8 calibrated offline (p^2-weighted rms rel-err ~1.7%) on the real score
# distribution; insensitive to round-vs-truncate in the f32->i16 convert.
EXP_A = 0.125 * np.log2(np.e) * 128.0
EXP_B = 16256.0 - 8.0


def build_kernel():
    nc = bacc.Bacc("TRN2", debug=False, enable_asserts=False, num_devices=N_CORES)

    # DRAM I/O (per-core shapes; identical program on every core)
    qT = nc.dram_tensor("qT", [D, S], BF16_T, kind="ExternalInput").ap()
    kT = nc.dram_tensor("kT", [D, S], BF16_T, kind="ExternalInput").ap()
    vT = nc.dram_tensor("vT", [D, S], BF16_T, kind="ExternalInput").ap()
    wqT2 = nc.dram_tensor("wqT2", [D, D2], BF16_T, kind="ExternalInput").ap()
    wkT2 = nc.dram_tensor("wkT2", [D, D2], BF16_T, kind="ExternalInput").ap()
    wvT2 = nc.dram_tensor("wvT2", [D, D2], BF16_T, kind="ExternalInput").ap()
    wo0 = nc.dram_tensor("wo0", [DH, D], BF16_T, kind="ExternalInput").ap()
    wo1 = nc.dram_tensor("wo1", [DH, D], BF16_T, kind="ExternalInput").ap()
    bq2 = nc.dram_tensor("bq2", [D2, 1], FP32, kind="ExternalInput").ap()
    bk2 = nc.dram_tensor("bk2", [D2, 1], FP32, kind="ExternalInput").ap()
    bv2 = nc.dram_tensor("bv2", [1, D2], BF16_T, kind="ExternalInput").ap()
    ouT = nc.dram_tensor("ouT", [D, S], BF16_T, kind="ExternalOutput").ap()

    KD = D // 128  # 4 contraction chunks of 128

    with tile.TileContext(nc) as tc:
        with (
            tc.tile_pool(name="persist", bufs=1) as pp,
            tc.tile_pool(name="stream", bufs=8) as ps,
            tc.tile_pool(name="ptpool", bufs=6) as ppt,
            tc.tile_pool(name="norm", bufs=3) as pn,
            tc.tile_pool(name="outs", bufs=4) as po,
            tc.tile_pool(name="psum", bufs=1, space="PSUM") as psum,
        ):
            # ---- constants / weights to SBUF ----
            wq_sb = pp.tile([128, KD, D2], BF16_T)
            wk_sb = pp.tile([128, KD, D2], BF16_T)
            wv_sb = pp.tile([128, KD, D2], BF16_T)
            nc.sync.dma_start(out=wq_sb, in_=wqT2.rearrange("(c p) m -> p c m", p=128))
            nc.scalar.dma_start(out=wk_sb, in_=wkT2.rearrange("(c p) m -> p c m", p=128))
            nc.sync.dma_start(out=wv_sb, in_=wvT2.rearrange("(c p) m -> p c m", p=128))
            wo_sb = [pp.tile([DH, D], BF16_T, tag=f"wo{h}", name=f"wo{h}") for h in range(HPC)]
            nc.sync.dma_start(out=wo_sb[0], in_=wo0)
            nc.sync.dma_start(out=wo_sb[1], in_=wo1)
            bq_sb = pp.tile([D2, 1], FP32, tag="bq")
            bk_sb = pp.tile([D2, 1], FP32, tag="bk")
            bv_sb = pp.tile([1, D2], BF16_T, tag="bv")
            nc.sync.dma_start(out=bq_sb, in_=bq2)
            nc.scalar.dma_start(out=bk_sb, in_=bk2)
            nc.sync.dma_start(out=bv_sb, in_=bv2)
            bv_bc = pp.tile([128, D2], FP32, tag="bv_bc")
            bv_f32 = pp.tile([1, D2], FP32, tag="bv_f32")
            nc.vector.tensor_copy(out=bv_f32, in_=bv_sb)
            nc.gpsimd.partition_broadcast(bv_bc, bv_f32)
            exp_src = pp.tile([1, 128], FP32, tag="exp_src")
            exp_dst = pp.tile([1, 128], FP32, tag="exp_dst")
            nc.vector.memset(exp_src, 0.0)
            nc.scalar.activation(out=exp_dst, in_=exp_src, func=AF.Exp, scale=1.0)

            # ---- persistent activations ----
            qpT = pp.tile([D2, S], BF16_T, tag="qpT")   # [2*dh, s] both heads
            kpT = pp.tile([D2, S], BF16_T, tag="kpT")
            # vp per head: [kv in chunk, chunk, dh+1]; last col = ones (denom)
            vp = [pp.tile([128, N_KVC, 128], BF16_T, tag=f"vp{h}", name=f"vp{h}") for h in range(HPC)]
            for h in range(HPC):
                nc.vector.memset(vp[h][:, :, DH + 1 :], 0.0)
                nc.vector.memset(vp[h][:, :, DH : DH + 1], 1.0)

            # ---- stage A: projections ----
            # qpT / kpT: out[j2, s] = W.T @ xT  (lhsT = w chunk, rhs = xT chunk)
            for name, src_, w_sb, b_sb, dst in (
                ("q", qT, wq_sb, bq_sb, qpT),
                ("k", kT, wk_sb, bk_sb, kpT),
            ):
                xin = [ps.tile([128, S], BF16_T, tag="xin", name=f"xin_{name}_{kc}") for kc in range(KD)]
                deng = nc.scalar if name == "k" else nc.sync
                for sb in range(4):
                    ss = slice(sb * (S // 4), (sb + 1) * (S // 4))
                    for kc in range(KD):
                        deng.dma_start(
                            out=xin[kc][:, ss], in_=src_[kc * 128 : (kc + 1) * 128, ss]
                        )
                for sc in range(S // 512):
                    pt = psum.tile([D2, 512], FP32, tag=("sA" if sc % 2 == 0 else "sB"), name="pt_qk")
                    for kc in range(KD):
                        nc.tensor.matmul(
                            pt,
                            w_sb[:, kc, :],
                            xin[kc][:, sc * 512 : (sc + 1) * 512],
                            start=(kc == 0),
                            stop=(kc == KD - 1),
                        )
                    # evict + per-partition bias on DVE (keeps ACT free for exp)
                    nc.vector.tensor_scalar_add(
                        out=dst[:, sc * 512 : (sc + 1) * 512],
                        in0=pt,
                        scalar1=b_sb,
                    )
            # vp: out[s, j2] = x @ W.T  (lhsT = vT chunk (stationary), rhs = w chunk)
            vin = [ps.tile([128, S], BF16_T, tag="xin", name=f"vin_{kc}") for kc in range(KD)]
            for sb in range(4):
                ss = slice(sb * (S // 4), (sb + 1) * (S // 4))
                for kc in range(KD):
                    nc.sync.dma_start(
                        out=vin[kc][:, ss], in_=vT[kc * 128 : (kc + 1) * 128, ss]
                    )
            def emit_vp():
                for sc in range(N_KVC):
                    pt = psum.tile([128, D2], FP32, tag=f"o{sc % 2}", name="pt_v")
                    for kc in range(KD):
                        nc.tensor.matmul(
                            pt,
                            vin[kc][:, sc * 128 : (sc + 1) * 128],
                            wv_sb[:, kc, :],
                            start=(kc == 0),
                            stop=(kc == KD - 1),
                        )
                    for h in range(HPC):
                        nc.vector.tensor_add(
                            out=vp[h][:, sc, 0:DH],
                            in0=pt[:, h * DH : (h + 1) * DH],
                            in1=bv_bc[:, h * DH : (h + 1) * DH],
                        )

            # ---- stage B: attention + output projection ----
            # Per event (qc, kv-group of glen chunks): 2*glen score slices of
            # [128, 512].  First `act` slices go to PSUM tile sA (exact exp on
            # ACT, two instructions), the rest to sB (Schraudolph exp on DVE).
            # Banks: sA 4 + sB 2 + o_tiles 2 = 8.
            groups = []
            kv = 0
            while kv < N_KVC:
                n = min(GROUP, N_KVC - kv)
                groups.append((kv, n))
                kv += n

            o_tiles = {}

            def make_attn(qc, g0, glen, p_sb):
                def emit():
                    for gi in range(glen):
                        kvc = g0 + gi
                        for h in range(HPC):
                            k = h * glen + gi
                            nc.tensor.matmul(
                                o_tiles[(qc, h)],
                                vp[h][:, kvc, :],
                                p_sb[:, k, :],
                                start=(kvc == 0),
                                stop=(kvc == N_KVC - 1),
                            )
                return emit

            def make_norm(qc):
                def emit():
                    ous = []
                    den2 = pn.tile([1, HPC * QC], FP32, tag="den2", name="den2")
                    for h in range(HPC):
                        ou = pn.tile([DH, QC], FP32, tag=f"ou{h}", name=f"ou{h}")
                        nc.vector.tensor_copy(out=ou, in_=o_tiles[(qc, h)][0:DH, :])
                        nc.vector.tensor_copy(
                            out=den2[0:1, h * QC : (h + 1) * QC],
                            in_=o_tiles[(qc, h)][DH : DH + 1, :],
                        )
                        ous.append(ou)
                    rec2 = pn.tile([1, HPC * QC], FP32, tag="rec2", name="rec2")
                    nc.vector.reciprocal_approx_fast(out=rec2, in_=den2)
                    outn = []
                    for h in range(HPC):
                        bcast = pn.tile([DH, QC], FP32, tag=f"bcast{h}", name=f"bcast{h}")
                        nc.gpsimd.partition_broadcast(
                            bcast, rec2[0:1, h * QC : (h + 1) * QC]
                        )
                        on = pn.tile([DH, QC], BF16_T, tag=f"outn{h}", name=f"on{h}")
                        nc.gpsimd.tensor_mul(on, ous[h], bcast)
                        outn.append(on)
                    return outn
                return emit

            def make_proj(qc, outn, tags=None):
                def emit():
                    qs = slice(qc * QC, (qc + 1) * QC)
                    for ec in range(D // 128):
                        tag = tags[ec] if tags else ("sA" if qc % 2 == 0 else "sB")
                        op = psum.tile([128, QC], FP32, tag=tag, name="op")
                        nc.tensor.matmul(
                            op, wo_sb[0][:, ec * 128 : (ec + 1) * 128], outn[0],
                            start=True, stop=False,
                        )
                        nc.tensor.matmul(
                            op, wo_sb[1][:, ec * 128 : (ec + 1) * 128], outn[1],
                            start=False, stop=True,
                        )
                        ot = po.tile([128, QC], BF16_T, tag="ot", name="ot")
                        nc.vector.tensor_copy(out=ot, in_=op)
                        nc.sync.dma_start(
                            out=ouT[ec * 128 : (ec + 1) * 128, qs], in_=ot
                        )
                return emit

            # software pipeline: attnV of each event is deferred one event so
            # the PE never sits behind the exp() it just fed; per-qc epilogue
            # (normalize + outproj) is deferred past that attnV.
            attn_q = []      # (qc, emit_fn, is_last_group_of_qc)
            norm_out = {}    # qc -> outn tiles
            proj_cd = None   # (countdown, qc)
            LAG = 2

            def pump(drain=False):
                nonlocal proj_cd
                while len(attn_q) > (0 if drain else LAG):
                    aqc, fn, last = attn_q.pop(0)
                    fn()
                    if last:
                        norm_out[aqc] = make_norm(aqc)()
                        proj_cd = [4, aqc]
                if proj_cd is not None:
                    if drain:
                        proj_cd[0] = 0
                    if proj_cd[0] <= 0:
                        pqc = proj_cd[1]
                        tags = ["o0", "o1", "o0", "o1"] if drain else None
                        make_proj(pqc, norm_out.pop(pqc), tags)()
                        proj_cd = None
                    else:
                        proj_cd[0] -= 1

            for qc in range(N_QC):
                qs = slice(qc * QC, (qc + 1) * QC)
                for h in range(HPC):
                    o_tiles[(qc, h)] = psum.tile(
                        [128, QC], FP32, tag=f"o{h}", name=f"o_ps{h}"
                    )
                for evi, (g0, glen) in enumerate(groups):
                    n_slices = HPC * glen
                    n_dve = n_slices // 3
                    n_act = n_slices - n_dve
                    sA = psum.tile([128, n_act, QC], FP32, tag="sA", name="sA")
                    sB = psum.tile([128, n_dve, QC], FP32, tag="sB", name="sB")
                    for h in range(HPC):
                        for gi in range(glen):
                            kvc = g0 + gi
                            k = h * glen + gi
                            hs = slice(h * DH, (h + 1) * DH)
                            dst = sA[:, k, :] if k < n_act else sB[:, k - n_act, :]
                            nc.tensor.matmul(
                                dst,
                                kpT[hs, kvc * KVC : (kvc + 1) * KVC],
                                qpT[hs, qs],
                                start=True,
                                stop=True,
                            )
                    p_sb = ppt.tile([128, n_slices, QC], BF16_T, tag="pt", name="p_sb")
                    # exact exp on ACT, split so PSUM banks release early
                    nc.scalar.activation(
                        out=p_sb[:, 0:2, :], in_=sA[:, 0:2, :], func=AF.Exp, scale=0.125
                    )
                    if n_act > 2:
                        nc.scalar.activation(
                            out=p_sb[:, 2:n_act, :], in_=sA[:, 2:n_act, :],
                            func=AF.Exp, scale=0.125,
                        )
                    # Schraudolph exp on DVE: int16(score*A + B) bitcast bf16
                    nc.vector.tensor_scalar(
                        out=p_sb[:, n_act:, :].bitcast(I16),
                        in0=sB,
                        scalar1=float(EXP_A),
                        scalar2=float(EXP_B),
                        op0=ALU.mult,
                        op1=ALU.add,
                    )
                    attn_q.append(
                        (qc, make_attn(qc, g0, glen, p_sb), g0 + glen == N_KVC)
                    )
                    if qc == 0 and evi < 3:
                        continue
                    if qc == 0 and evi == 3:
                        emit_vp()
                    pump()
            pump(drain=True)
    nc.compile()
    return nc


_NC_CACHE = None


def _get_nc():
    global _NC_CACHE
    if _NC_CACHE is None:
        _NC_CACHE = build_kernel()
    return _NC_CACHE


def make_in_maps(q, k, v, w_q, b_q, w_k, b_k, w_v, b_v, w_o, b_o):
    """Shard the full inputs into the 8 per-core input maps."""
    q = np.asarray(q, np.float32)
    k = np.asarray(k, np.float32)
    v = np.asarray(v, np.float32)
    w_q = np.asarray(w_q, np.float32)
    w_k = np.asarray(w_k, np.float32)
    w_v = np.asarray(w_v, np.float32)
    w_o = np.asarray(w_o, np.float32)
    b_q = np.asarray(b_q, np.float32)
    b_k = np.asarray(b_k, np.float32)
    b_v = np.asarray(b_v, np.float32)

    qT = [np.ascontiguousarray(q[b].T).astype(BF16) for b in range(B)]
    kTb = [np.ascontiguousarray(k[b].T).astype(BF16) for b in range(B)]
    vTb = [np.ascontiguousarray(v[b].T).astype(BF16) for b in range(B)]
    wqT = np.ascontiguousarray(w_q.T).astype(BF16)  # [D, D] = [d, j]
    wkT = np.ascontiguousarray(w_k.T).astype(BF16)
    wvT = np.ascontiguousarray(w_v.T).astype(BF16)

    in_maps = []
    for c in range(N_CORES):
        b = c // 4
        hp = c % 4
        js = slice(hp * D2, (hp + 1) * D2)
        h0 = hp * D2
        in_maps.append(
            {
                "qT": qT[b],
                "kT": kTb[b],
                "vT": vTb[b],
                "wqT2": np.ascontiguousarray(wqT[:, js]),
                "wkT2": np.ascontiguousarray(wkT[:, js]),
                "wvT2": np.ascontiguousarray(wvT[:, js]),
                "wo0": np.ascontiguousarray(w_o[:, h0 : h0 + DH].T).astype(BF16),
                "wo1": np.ascontiguousarray(w_o[:, h0 + DH : h0 + 2 * DH].T).astype(BF16),
                "bq2": np.ascontiguousarray(b_q[js].reshape(D2, 1)),
                "bk2": np.ascontiguousarray(b_k[js].reshape(D2, 1)),
                "bv2": np.ascontiguousarray(b_v[js].reshape(1, D2)).astype(BF16),
            }
        )
    return in_maps


def gather_output(results, b_o):
    """Sum per-batch partials, add output bias, restore [B, S, D] layout."""
    b_o = np.asarray(b_o, np.float32)
    out = np.empty((B, S, D), np.float32)
    for b in range(B):
        acc = np.zeros((D, S), np.float32)
        for c in range(b * 4, b * 4 + 4):
            acc += results[c]["ouT"].astype(np.float32)
        out[b] = acc.T + b_o[None, :]
    return out


def kernel(q, k, v, w_q, b_q, w_k, b_k, w_v, b_v, w_o, b_o):
    nc = _get_nc()
    in_maps = make_in_maps(q, k, v, w_q, b_q, w_k, b_k, w_v, b_v, w_o, b_o)
    res = run_bass_kernel_spmd(nc, in_maps, core_ids=list(range(N_CORES)))
    return gather_output(res.results, b_o)


# revision 11
# speedup vs baseline: 1.0665x; 1.0665x over previous
"""Multi-head attention (B=2, S=4096, D=512, H=8) on 8 Trainium2 NeuronCores.

Sharding: batch x head-pair.  Core c handles batch b = c//4 and heads
(2*(c%4), 2*(c%4)+1).  Each core computes its heads' Q/K/V projections,
flash-style attention (scores kept transposed [kv, q]), and its heads' slice
of the output projection; the 4 per-batch partials are summed on the host.

Key engine-level structure:
- Score matmuls have K=64 (head dim), so the two heads' matmuls are emitted
  pair-adjacent: bass auto-derives tile_position (0,0)/(64,0) from the operand
  partitions and the PE array runs both 64x128 tiles concurrently (~2x).
- exp() is split across engines per event: DVE computes the gi0 pair (+ gi2/h1)
  with a one-instruction Schraudolph bit-trick (scale+bias into int16, bitcast
  bf16); ACT computes the rest exactly.  Softmax normalization cancels the
  correlated part of the approximation error (measured ~9.7e-3 end to end).
- Softmax denominators ride as an extra ones-column in each head's V tile
  (col 64 for h0, col 63 for h1) so they land on distinct, partition-aligned
  PSUM rows; normalization is a partition-aligned reciprocal + gpsimd
  broadcast + one Pool multiply.
- V's bias is folded into the host-side output bias (softmax weights sum to 1,
  so attn(v + b_v) = attn(v) + b_v).
- Output projection contracts over both heads at once (K=128) producing
  [q, d] tiles DMA'd straight to a [S, D] output.
"""

import sys

sys.path.insert(0, "/opt/trn_rl_repo")

import numpy as np
import ml_dtypes

import concourse.bacc as bacc
import concourse.bass as bass
import concourse.tile as tile
from concourse import mybir
from concourse.bass_utils import run_bass_kernel_spmd

BF16 = ml_dtypes.bfloat16

B = 2
S = 4096
D = 512
H = 8
DH = 64           # head dim
HPC = 2           # heads per core
D2 = HPC * DH     # 128, the two heads' feature slice
N_CORES = 8
QC = 512          # query chunk (free dim of scores/attnV matmuls)
KVC = 128         # kv chunk (partition dim of transposed scores)
N_QC = S // QC    # 8
N_KVC = S // KVC  # 32
GROUP = 3         # kv chunks per event

# v-projection chunks emitted per qc0 event (event -> sc range)
VP_SCHED = {3: (0, 6), 4: (6, 12), 5: (12, 16), 6: (16, 20),
            7: (20, 24), 8: (24, 28), 9: (28, 32)}

FP32 = mybir.dt.float32
BF16_T = mybir.dt.bfloat16
I16 = mybir.dt.int16
AF = mybir.ActivationFunctionType
ALU = mybir.AluOpType

# Schraudolph exp constants for the DVE path: bitcast_bf16(int16(
#   score * (0.125*log2(e)*128) + (127*128 - C))) ~= exp(score * 0.125).
# C=8 calibrated offline (p^2-weighted rms rel-err ~1.7%) on the real score
# distribution; insensitive to round-vs-truncate in the f32->i16 convert.
EXP_A = 0.125 * np.log2(np.e) * 128.0
EXP_B = 16256.0 - 8.0


def build_kernel():
    nc = bacc.Bacc("TRN2", debug=False, enable_asserts=False, num_devices=N_CORES)

    # DRAM I/O (per-core shapes; identical program on every core)
    qT = nc.dram_tensor("qT", [D, S], BF16_T, kind="ExternalInput").ap()
    kT = nc.dram_tensor("kT", [D, S], BF16_T, kind="ExternalInput").ap()
    vT = nc.dram_tensor("vT", [D, S], BF16_T, kind="ExternalInput").ap()
    wqT2 = nc.dram_tensor("wqT2", [D, D2], BF16_T, kind="ExternalInput").ap()
    wkT2 = nc.dram_tensor("wkT2", [D, D2], BF16_T, kind="ExternalInput").ap()
    wvT2 = nc.dram_tensor("wvT2", [D, D2], BF16_T, kind="ExternalInput").ap()
    woT2 = nc.dram_tensor("woT2", [D2, D], BF16_T, kind="ExternalInput").ap()
    bq2 = nc.dram_tensor("bq2", [D2, 1], FP32, kind="ExternalInput").ap()
    bk2 = nc.dram_tensor("bk2", [D2, 1], FP32, kind="ExternalInput").ap()
    ouT = nc.dram_tensor("ouT", [S, D], BF16_T, kind="ExternalOutput").ap()

    KD = D // 128   # 4 contraction chunks of 128
    NSB = 8         # DMA chunks per input tensor (512 cols each)
    SBW = S // NSB

    with tile.TileContext(nc) as tc:
        with (
            tc.tile_pool(name="persist", bufs=1) as pp,
            tc.tile_pool(name="stream", bufs=8) as ps,
            tc.tile_pool(name="ptpool", bufs=6) as ppt,
            tc.tile_pool(name="norm", bufs=2) as pn,
            tc.tile_pool(name="outs", bufs=4) as po,
            tc.tile_pool(name="psum", bufs=1, space="PSUM") as psum,
        ):
            # ---- constants / weights to SBUF ----
            wq_sb = pp.tile([128, KD, D2], BF16_T)
            wk_sb = pp.tile([128, KD, D2], BF16_T)
            wv_sb = pp.tile([128, KD, D2], BF16_T)
            nc.sync.dma_start(out=wq_sb, in_=wqT2.rearrange("(c p) m -> p c m", p=128))
            nc.scalar.dma_start(out=wk_sb, in_=wkT2.rearrange("(c p) m -> p c m", p=128))
            nc.scalar.dma_start(out=wv_sb, in_=wvT2.rearrange("(c p) m -> p c m", p=128))
            wo_sb = pp.tile([D2, D], BF16_T, tag="wo")
            nc.sync.dma_start(out=wo_sb, in_=woT2)
            bq_sb = pp.tile([D2, 1], FP32, tag="bq")
            bk_sb = pp.tile([D2, 1], FP32, tag="bk")
            nc.sync.dma_start(out=bq_sb, in_=bq2)
            nc.scalar.dma_start(out=bk_sb, in_=bk2)
            exp_src = pp.tile([1, 128], FP32, tag="exp_src")
            exp_dst = pp.tile([1, 128], FP32, tag="exp_dst")
            nc.vector.memset(exp_src, 0.0)
            nc.scalar.activation(out=exp_dst, in_=exp_src, func=AF.Exp, scale=1.0)

            # ---- persistent activations ----
            qpT = pp.tile([D2, S], BF16_T, tag="qpT")   # [2*dh, s] both heads
            kpT = pp.tile([D2, S], BF16_T, tag="kpT")
            # vp per head: [kv in chunk, chunk, 128].  h0: V dims in cols 0:64,
            # ones col 64 (denominator row 64 of o0).  h1: V dims in cols
            # 64:128, ones col 0 (denominator row 0 of o1 — partition offsets
            # must be 32-aligned for engine access).
            vp = [pp.tile([128, N_KVC, 128], BF16_T, tag=f"vp{h}", name=f"vp{h}") for h in range(HPC)]
            nc.vector.memset(vp[0][:, :, DH + 1 :], 0.0)
            nc.vector.memset(vp[0][:, :, DH : DH + 1], 1.0)
            nc.vector.memset(vp[1][:, :, 1:DH], 0.0)
            nc.vector.memset(vp[1][:, :, 0:1], 1.0)

            # ---- input DMAs: q on sync queue, k on scalar, v interleaved ----
            xq = [ps.tile([128, S], BF16_T, tag="xin", name=f"xq_{kc}") for kc in range(KD)]
            xk = [ps.tile([128, S], BF16_T, tag="xin", name=f"xk_{kc}") for kc in range(KD)]
            vin = [ps.tile([128, S], BF16_T, tag="vin", bufs=4, name=f"vin_{kc}") for kc in range(KD)]

            def dma_chunk(eng, dst_tiles, src, sb):
                ss = slice(sb * SBW, (sb + 1) * SBW)
                for kc in range(KD):
                    eng.dma_start(
                        out=dst_tiles[kc][:, ss], in_=src[kc * 128 : (kc + 1) * 128, ss]
                    )

            # first few q/k chunks up front; v rides along afterwards
            for sb in range(3):
                dma_chunk(nc.sync, xq, qT, sb)
                dma_chunk(nc.scalar, xk, kT, sb)
            for sb in range(3, NSB):
                dma_chunk(nc.sync, xq, qT, sb)
                dma_chunk(nc.sync, vin, vT, sb - 3)
                dma_chunk(nc.scalar, xk, kT, sb)
            for sb in range(NSB - 3, NSB):
                dma_chunk(nc.scalar, vin, vT, sb)

            # ---- stage A: q/k projections  out[j2, s] = W.T @ xT ----
            for name, xin, w_sb, b_sb, dst in (
                ("q", xq, wq_sb, bq_sb, qpT),
                ("k", xk, wk_sb, bk_sb, kpT),
            ):
                for sc in range(NSB):
                    pt = psum.tile(
                        [D2, SBW], FP32,
                        tag=("sA" if sc % 2 == 0 else "sB"), name="pt_qk",
                    )
                    for kc in range(KD):
                        nc.tensor.matmul(
                            pt,
                            w_sb[:, kc, :],
                            xin[kc][:, sc * SBW : (sc + 1) * SBW],
                            start=(kc == 0),
                            stop=(kc == KD - 1),
                        )
                    nc.vector.tensor_scalar_add(
                        out=dst[:, sc * SBW : (sc + 1) * SBW],
                        in0=pt,
                        scalar1=b_sb,
                    )

            # ---- v projection (emitted spread across early events) ----
            # out[s, j2] = x @ W.T; evict h0 cols to vp0[:, :, 0:64] (DVE) and
            # h1 cols to vp1[:, :, 64:128] (ACT).  No bias (folded into b_o on
            # the host).
            def emit_vp(sc):
                pt = psum.tile(
                    [128, D2], FP32,
                    tag=("sA" if sc % 2 == 0 else "sB"), name="pt_v",
                )
                for kc in range(KD):
                    nc.tensor.matmul(
                        pt,
                        vin[kc][:, sc * 128 : (sc + 1) * 128],
                        wv_sb[:, kc, :],
                        start=(kc == 0),
                        stop=(kc == KD - 1),
                    )
                nc.vector.tensor_copy(out=vp[0][:, sc, 0:DH], in_=pt[:, 0:DH])
                nc.scalar.copy(out=vp[1][:, sc, DH:], in_=pt[:, DH:])

            # ---- stage B: attention + output projection ----
            # Event = (qc, kv-group of glen chunks): 2*glen score slices of
            # [128, 512], written pair-major so the two heads' K=64 matmuls
            # overlap in the PE array.  PSUM slot map (glen=3):
            #   sB: [gi0/h0, gi0/h1, gi2/h1]  (DVE Schraudolph exp)
            #   sA: [gi1/h0, gi1/h1, gi2/h0]  (ACT exact exp)
            # p slot = 2*gi + h.
            groups = []
            kv = 0
            while kv < N_KVC:
                n = min(GROUP, N_KVC - kv)
                groups.append((kv, n))
                kv += n

            o_tiles = {}

            def make_attn(qc, g0, glen, p_sb):
                def emit():
                    for gi in range(glen):
                        kvc = g0 + gi
                        for h in range(HPC):
                            nc.tensor.matmul(
                                o_tiles[(qc, h)],
                                vp[h][:, kvc, :],
                                p_sb[:, 2 * gi + h, :],
                                start=(kvc == 0),
                                stop=(kvc == N_KVC - 1),
                            )
                return emit

            def make_norm(qc):
                def emit():
                    # h0 out dims at o0 rows 0:64 (den row 64); h1 at o1 rows
                    # 64:128 (den row 0).  Denominators are copied down to
                    # partition 0 (engines support a partition base-shift on
                    # reads), reciprocal'd there, and broadcast full-width so
                    # every multiply operand is offset-aligned.
                    oup = pn.tile([128, QC], FP32, tag="oup", name="oup")
                    den2 = pn.tile([1, 2 * QC], FP32, tag="den2", name="den2")
                    nc.vector.tensor_copy(out=oup[0:DH, :], in_=o_tiles[(qc, 0)][0:DH, :])
                    nc.scalar.copy(out=oup[DH:, :], in_=o_tiles[(qc, 1)][DH:, :])
                    nc.vector.tensor_copy(
                        out=den2[0:1, 0:QC], in_=o_tiles[(qc, 0)][DH : DH + 1, :]
                    )
                    nc.scalar.copy(out=den2[0:1, QC:], in_=o_tiles[(qc, 1)][0:1, :])
                    rec2 = pn.tile([1, 2 * QC], FP32, tag="rec2", name="rec2")
                    nc.vector.reciprocal_approx_fast(out=rec2, in_=den2)
                    bc0 = pn.tile([128, QC], FP32, tag="bc0", name="bc0")
                    bc1 = pn.tile([128, QC], FP32, tag="bc1", name="bc1")
                    nc.gpsimd.partition_broadcast(bc0, rec2[0:1, 0:QC])
                    nc.gpsimd.partition_broadcast(bc1, rec2[0:1, QC:])
                    outn = pn.tile([128, QC], BF16_T, tag="outn", name="outn")
                    nc.gpsimd.tensor_mul(outn[0:DH, :], oup[0:DH, :], bc0[0:DH, :])
                    nc.gpsimd.tensor_mul(outn[DH:, :], oup[DH:, :], bc1[DH:, :])
                    return outn
                return emit

            def make_proj(qc, outn, drain=False):
                def emit():
                    for qq in range(QC // 128):
                        if drain:
                            tag = "o0" if qq % 2 == 0 else "o1"
                        else:
                            tag = "sA" if qq % 2 == 0 else "sB"
                        op = psum.tile([128, D], FP32, tag=tag, name="op")
                        nc.tensor.matmul(
                            op, outn[:, qq * 128 : (qq + 1) * 128], wo_sb,
                            start=True, stop=True,
                        )
                        ot = po.tile([128, D], BF16_T, tag="ot", name="ot")
                        if qq % 2 == 0:
                            nc.vector.tensor_copy(out=ot, in_=op)
                        else:
                            nc.scalar.copy(out=ot, in_=op)
                        rs = qc * QC + qq * 128
                        nc.sync.dma_start(out=ouT[rs : rs + 128, :], in_=ot)
                return emit

            # software pipeline: attnV of each event is deferred LAG events;
            # per-qc epilogue (normalize + outproj) is deferred past that.
            attn_q = []      # (qc, emit_fn, is_last_group_of_qc)
            norm_out = {}    # qc -> outn tile
            proj_cd = None   # (countdown, qc)
            LAG = 2

            def pump(drain=False):
                nonlocal proj_cd
                while len(attn_q) > (0 if drain else LAG):
                    aqc, fn, last = attn_q.pop(0)
                    fn()
                    if last:
                        norm_out[aqc] = make_norm(aqc)()
                        proj_cd = [4, aqc]
                if proj_cd is not None:
                    if drain:
                        proj_cd[0] = 0
                    if proj_cd[0] <= 0:
                        pqc = proj_cd[1]
                        make_proj(pqc, norm_out.pop(pqc), drain)()
                        proj_cd = None
                    else:
                        proj_cd[0] -= 1

            for qc in range(N_QC):
                qs = slice(qc * QC, (qc + 1) * QC)
                for h in range(HPC):
                    o_tiles[(qc, h)] = psum.tile(
                        [128, QC], FP32, tag=f"o{h}", name=f"o_ps{h}"
                    )
                for evi, (g0, glen) in enumerate(groups):
                    n_slices = HPC * glen
                    n_dve = n_slices // 2
                    n_act = n_slices - n_dve
                    sA = psum.tile([128, n_act, QC], FP32, tag="sA", name="sA")
                    sB = psum.tile([128, n_dve, QC], FP32, tag="sB", name="sB")
                    p_sb = ppt.tile([128, n_slices, QC], BF16_T, tag="pt", name="p_sb")

                    # (gi, h) -> (dst tile, slot): gi0 pair -> sB[0:2],
                    # gi1 pair -> sA[0:2], gi2 -> h0: sA[2], h1: sB[2]
                    def score_dst(gi, h):
                        if gi == 0:
                            return sB[:, h, :]
                        if gi == 1:
                            return sA[:, h, :]
                        return sA[:, 2, :] if h == 0 else sB[:, 2, :]

                    for gi in range(glen):
                        kvc = g0 + gi
                        for h in range(HPC):
                            hs = slice(h * DH, (h + 1) * DH)
                            nc.tensor.matmul(
                                score_dst(gi, h),
                                kpT[hs, kvc * KVC : (kvc + 1) * KVC],
                                qpT[hs, qs],
                                start=True,
                                stop=True,
                            )
                    # exp: DVE gets sB (Schraudolph), ACT gets sA (exact).
                    # p slot = 2*gi + h: sB slots = [0, 1, 5]; sA = [2, 3, 4].
                    nc.vector.tensor_scalar(
                        out=p_sb[:, 0:2, :].bitcast(I16),
                        in0=sB[:, 0:2, :],
                        scalar1=float(EXP_A),
                        scalar2=float(EXP_B),
                        op0=ALU.mult,
                        op1=ALU.add,
                    )
                    nc.scalar.activation(
                        out=p_sb[:, 2:4, :], in_=sA[:, 0:2, :], func=AF.Exp, scale=0.125
                    )
                    if glen == 3:
                        nc.scalar.activation(
                            out=p_sb[:, 4:5, :], in_=sA[:, 2:3, :],
                            func=AF.Exp, scale=0.125,
                        )
                        nc.vector.tensor_scalar(
                            out=p_sb[:, 5:6, :].bitcast(I16),
                            in0=sB[:, 2:3, :],
                            scalar1=float(EXP_A),
                            scalar2=float(EXP_B),
                            op0=ALU.mult,
                            op1=ALU.add,
                        )
                    attn_q.append(
                        (qc, make_attn(qc, g0, glen, p_sb), g0 + glen == N_KVC)
                    )
                    if qc == 0 and evi < 3:
                        continue
                    # v-projection spread across early events; must stay ahead
                    # of the attnV pops (pump pops ev0+ev1 at evi==3, then one
                    # event per pump; attnV of event e consumes vp sc 3e..3e+2).
                    if qc == 0 and evi in VP_SCHED:
                        for sc in range(*VP_SCHED[evi]):
                            emit_vp(sc)
                    pump()
            pump(drain=True)
    nc.compile()
    return nc


_NC_CACHE = None


def _get_nc():
    global _NC_CACHE
    if _NC_CACHE is None:
        _NC_CACHE = build_kernel()
    return _NC_CACHE


def make_in_maps(q, k, v, w_q, b_q, w_k, b_k, w_v, b_v, w_o, b_o):
    """Shard the full inputs into the 8 per-core input maps."""
    q = np.asarray(q, np.float32)
    k = np.asarray(k, np.float32)
    v = np.asarray(v, np.float32)
    w_q = np.asarray(w_q, np.float32)
    w_k = np.asarray(w_k, np.float32)
    w_v = np.asarray(w_v, np.float32)
    w_o = np.asarray(w_o, np.float32)
    b_q = np.asarray(b_q, np.float32)
    b_k = np.asarray(b_k, np.float32)

    qT = [np.ascontiguousarray(q[b].T).astype(BF16) for b in range(B)]
    kTb = [np.ascontiguousarray(k[b].T).astype(BF16) for b in range(B)]
    vTb = [np.ascontiguousarray(v[b].T).astype(BF16) for b in range(B)]
    wqT = np.ascontiguousarray(w_q.T).astype(BF16)  # [D, D] = [d, j]
    wkT = np.ascontiguousarray(w_k.T).astype(BF16)
    wvT = np.ascontiguousarray(w_v.T).astype(BF16)

    in_maps = []
    for c in range(N_CORES):
        b = c // 4
        hp = c % 4
        js = slice(hp * D2, (hp + 1) * D2)
        in_maps.append(
            {
                "qT": qT[b],
                "kT": kTb[b],
                "vT": vTb[b],
                "wqT2": np.ascontiguousarray(wqT[:, js]),
                "wkT2": np.ascontiguousarray(wkT[:, js]),
                "wvT2": np.ascontiguousarray(wvT[:, js]),
                "woT2": np.ascontiguousarray(w_o[:, js].T).astype(BF16),
                "bq2": np.ascontiguousarray(b_q[js].reshape(D2, 1)),
                "bk2": np.ascontiguousarray(b_k[js].reshape(D2, 1)),
            }
        )
    return in_maps


def gather_output(results, b_o, b_v, w_o):
    """Sum per-batch partials and add the folded output bias.

    V's bias is not applied on device; softmax weights sum to 1, so it
    shifts attention output by b_v, contributing b_v @ w_o.T after the
    output projection.
    """
    b_o = np.asarray(b_o, np.float32)
    b_v = np.asarray(b_v, np.float32)
    w_o = np.asarray(w_o, np.float32)
    bias = b_o + b_v @ w_o.T
    out = np.empty((B, S, D), np.float32)
    for b in range(B):
        acc = np.zeros((S, D), np.float32)
        for c in range(b * 4, b * 4 + 4):
            acc += results[c]["ouT"].astype(np.float32)
        out[b] = acc + bias[None, :]
    return out


def kernel(q, k, v, w_q, b_q, w_k, b_k, w_v, b_v, w_o, b_o):
    nc = _get_nc()
    in_maps = make_in_maps(q, k, v, w_q, b_q, w_k, b_k, w_v, b_v, w_o, b_o)
    res = run_bass_kernel_spmd(nc, in_maps, core_ids=list(range(N_CORES)))
    return gather_output(res.results, b_o, b_v, w_o)
